# revision 11
# baseline (speedup 1.0000x reference)
"""EngramEmbeddings Trainium2 kernel — unified sorted-gather design.

Expert-sharded across 8 NeuronCores: core c owns head c of the n=2 and n=3
hash tables and serves all 32768 tokens for its two slots (65536 lookups).

Design:
  The n2 (6689-row) and n3 (65579-row) tables are concatenated into one
  merged table of 72268 rows, each padded to 512B (128 f32).  All 65536
  lookups (32768 n2 + 32768 n3) form one stream, HOST-sorted by index
  value into 16 chunks of 4096 whose index span fits int16 after
  subtracting a fixed per-chunk base (the sorted halves span ~840/~8.2k
  per chunk, far under 32768).  The host computes hashes ONLY to choose
  this permutation + verify the bases; the device recomputes the exact
  hash itself.

  Device phases (serialized on purpose — SWDGE descriptor generation
  starves the DVE of SBUF ports, so overlapping hash with gathers
  stretches vector ops up to ~100x):
    1. hash: unified 3-term hash (n2 entries use id0=0 so term0 == 0,
       XOR identity) over 4 slabs of 128 columns, with per-position
       seed/mod-constant planes; exact int64 semantics via 16-bit limb
       arithmetic on the vector engine; the four R-piece mods run as one
       4-wide batched mod.  += adj plane -> int16 idx in [0, 32768).
    2. gather: per chunk, two SBUF->SBUF copies scramble idx into the
       dma_gather wrapped stream layout for queue q's tx/rx partition
       groups (partitions 32q..32q+32), then dma_gather on queues 0-3
       round-robin (4 queues -> ~2ns/row of GpSimd desc-gen, the 4
       tx/rx Q7 core pairs all busy).  A dummy sync-engine copy after
       the last slab orders every stg copy (hence gather) behind the
       full hash.
    3. store: HWDGE (scalar-engine queue) writes the first 80 f32 of
       each gathered 128-f32 row -> outM[4096a..].

Host does sharding-style prep: dtype casts, the sort permutation and its
inverse, per-position constant planes, table concat/pad, final unpermute.
"""

import numpy as np

try:
    import concourse  # noqa: F401
except ImportError:  # pragma: no cover
    import sys

    for _p in ("/opt/trn_rl_repo", "/root/.axon_site/_ro/trn_rl_repo"):
        if _p not in sys.path:
            sys.path.insert(0, _p)

import concourse.tile as tile
from concourse import bacc, mybir
from concourse.bass_utils import run_bass_kernel_spmd

N2_SIZES = [6619, 6637, 6653, 6659, 6661, 6673, 6679, 6689]
N3_SIZES = [65521, 65537, 65539, 65543, 65551, 65557, 65563, 65579]
B, S = 8, 4096
P = 128
NTOK = B * S                  # 32768
NE = 2 * NTOK                 # 65536 lookups (n2 + n3)
COLS = NE // P                # 512 hash columns per partition
V2 = max(N2_SIZES)            # 6689
V3 = max(N3_SIZES)            # 65579
VM = V2 + V3                  # 72268 merged rows
EM = 128                      # merged row padded to 128 f32 = 512B
SLOT = 80
NCHUNK = 16                   # dma_gather chunks
CPC = NE // NCHUNK            # 4096 lookups per chunk
CW = CPC // P                 # 32 hash cols / sbuf row-blocks per chunk
SLABW = 128                   # hash slab width = 4 chunks
NSLAB = COLS // SLABW         # 4
CPS = SLABW // CW             # 4 chunks per slab

# fixed per-chunk bases: the sorted merged stream is exactly half n2
# (indices in [0, V2)) then half n3 ([V2, VM)), so chunks 0-7 are n2 and
# 8-15 n3; within each half chunks span ~1/8 of the range.  The -2048
# slack absorbs order-statistic fluctuation; host asserts every
# idx-base lands in [0, 32768) and recompiles with exact bases if not.
DEFAULT_BASES = tuple(
    max(0, (V2 * k) // 8 - 2048) if k < 8
    else max(0, V2 + (V3 * (k - 8)) // 8 - 2048)
    for k in range(NCHUNK)
)

_NC = {}
TRACE = False
LAST_RESULT = None


def _build_nc(bases):
    dt = mybir.dt
    A = mybir.AluOpType
    AND, XOR = A.bitwise_and, A.bitwise_xor
    LSR, LSL = A.logical_shift_right, A.logical_shift_left
    ADD, MULT, SUB, GE = A.add, A.mult, A.subtract, A.is_ge
    i32 = dt.int32
    f32 = dt.float32

    nc = bacc.Bacc("TRN2", target_bir_lowering=False, debug=False,
                   num_swdge_queues=4)
    tblM = nc.dram_tensor("tblM", [VM, EM], f32, kind="ExternalInput")
    idsd = nc.dram_tensor("ids", [3, NE], i32, kind="ExternalInput")
    # seed planes, slab-blocked: slab sb occupies cols [3*SLABW*sb,
    # 3*SLABW*(sb+1)) as [term0 | term1 | term2] blocks of SLABW
    s0d = nc.dram_tensor("s0w", [P, 3 * COLS], i32, kind="ExternalInput")
    s1d = nc.dram_tensor("s1w", [P, 3 * COLS], i32, kind="ExternalInput")
    # per-position mod constants: M, R16, R24, R32, R40 (int32) + inv (f32)
    cstd = nc.dram_tensor("cst", [P, 5 * COLS], i32, kind="ExternalInput")
    invd = nc.dram_tensor("inv", [P, COLS], f32, kind="ExternalInput")
    # 4x column-replicated M / inv for the batched piece-mod, slab-blocked:
    # slab sb at [4*SLABW*sb, 4*SLABW*(sb+1)), four identical SLABW blocks
    m4d = nc.dram_tensor("m4", [P, 4 * COLS], i32, kind="ExternalInput")
    i4d = nc.dram_tensor("i4", [P, 4 * COLS], f32, kind="ExternalInput")
    adjd = nc.dram_tensor("adj", [P, COLS], i32, kind="ExternalInput")
    outMd = nc.dram_tensor("outM", [NE, SLOT], f32, kind="ExternalOutput")

    with tile.TileContext(nc) as tc:
        with (
            tc.tile_pool(name="c", bufs=1) as cp,
            tc.tile_pool(name="w", bufs=1) as wp,
            tc.tile_pool(name="g", bufs=1) as gp,
        ):

            def ld(dram, shape, dtype, tag):
                t = cp.tile(shape, dtype, tag=tag, name=tag)
                nc.sync.dma_start(t[:], dram.ap())
                return t

            s0w = ld(s0d, [P, 3 * COLS], i32, "s0w")
            s1w = ld(s1d, [P, 3 * COLS], i32, "s1w")
            cst = ld(cstd, [P, 5 * COLS], i32, "cst")
            inv = ld(invd, [P, COLS], f32, "inv")
            m4 = ld(m4d, [P, 4 * COLS], i32, "m4")
            i4 = ld(i4d, [P, 4 * COLS], f32, "i4")
            adj = ld(adjd, [P, COLS], i32, "adj")

            idsv = idsd.ap().rearrange("r (p c) -> r p c", p=P)
            ids = []
            for r in range(3):
                t_ = cp.tile([P, COLS], i32, tag=f"id{r}", name=f"id{r}")
                nc.sync.dma_start(t_[:], idsv[r])
                ids.append(t_)

            outMv = outMd.ap().rearrange("(k p b) d -> k p b d", p=P, b=CW)

            def hash_slab(sb):
                """int16 gather indices for slab sb (4 chunks, 128 cols).

                Exact int64 hash via 16-bit limbs (DVE fp32-internal ops
                stay < 2^24; bit surgery uses exact int32 bitwise/shifts;
                mod is reciprocal-multiply + floor + conditional subtract).
                """
                C = SLABW
                W = 3 * C
                col0 = C * sb

                def wt():
                    return wp.tile([P, W], i32, tag="wm", bufs=14,
                                   name=f"wm_{nc.next_id()}")

                def st(dtype=i32):
                    return wp.tile([P, C], dtype, tag=f"sm{dtype}",
                                   bufs=10, name=f"sm_{nc.next_id()}")

                sl = [slice(j * C, (j + 1) * C) for j in range(3)]
                cs = slice(col0, col0 + C)
                Mt = cst[:, 0 * COLS + col0 : 0 * COLS + col0 + C]
                R16 = cst[:, 1 * COLS + col0 : 1 * COLS + col0 + C]
                R24 = cst[:, 2 * COLS + col0 : 2 * COLS + col0 + C]
                R32 = cst[:, 3 * COLS + col0 : 3 * COLS + col0 + C]
                R40 = cst[:, 4 * COLS + col0 : 4 * COLS + col0 + C]
                INV = inv[:, cs]
                s0v = s0w[:, 3 * col0 : 3 * col0 + W]
                s1v = s1w[:, 3 * col0 : 3 * col0 + W]
                M4 = m4[:, 4 * col0 : 4 * col0 + 4 * C]
                I4 = i4[:, 4 * col0 : 4 * col0 + 4 * C]

                X = wt()
                for j, src in enumerate(ids):
                    nc.scalar.copy(X[:, sl[j]], src[:, cs])
                a0 = wt()
                nc.vector.tensor_scalar(a0[:], X[:], 0xFF, None, AND)
                a1 = wt()
                nc.vector.tensor_scalar(a1[:], X[:], 8, None, LSR)
                t00, t10, t01, t11 = wt(), wt(), wt(), wt()
                nc.vector.tensor_tensor(t00[:], a0[:], s0v, MULT)
                nc.vector.tensor_tensor(t10[:], a1[:], s0v, MULT)
                nc.vector.tensor_tensor(t01[:], a0[:], s1v, MULT)
                nc.vector.tensor_tensor(t11[:], a1[:], s1v, MULT)
                Apt = wt()
                nc.vector.tensor_scalar(Apt[:], t10[:], 0xFF, 8, AND, LSL)
                v0a = wt()
                nc.vector.tensor_scalar(v0a[:], t00[:], 0xFFFF, None, AND)
                v0 = wt()
                nc.vector.tensor_tensor(v0[:], v0a[:], Apt[:], ADD)
                L0 = wt()
                nc.vector.tensor_scalar(L0[:], v0[:], 0xFFFF, None, AND)
                c0 = wt()
                nc.vector.tensor_scalar(c0[:], v0[:], 16, None, LSR)
                u1a = wt()
                nc.vector.tensor_scalar(u1a[:], t10[:], 8, None, LSR)
                u1 = wt()
                nc.vector.tensor_tensor(u1[:], u1a[:], c0[:], ADD)
                u2a = wt()
                nc.vector.tensor_scalar(u2a[:], t01[:], 0xFFFF, None, AND)
                u2 = wt()
                nc.vector.tensor_tensor(u2[:], u2a[:], u1[:], ADD)
                u3a = wt()
                nc.vector.tensor_scalar(u3a[:], t00[:], 16, None, LSR)
                v1 = wt()
                nc.vector.tensor_tensor(v1[:], u3a[:], u2[:], ADD)
                Ff = wt()
                nc.vector.tensor_scalar(Ff[:], t11[:], 0xFF, 8, AND, LSL)
                v1b = wt()
                nc.vector.tensor_tensor(v1b[:], v1[:], Ff[:], ADD)
                L1 = wt()
                nc.vector.tensor_scalar(L1[:], v1b[:], 0xFFFF, None, AND)
                c1 = wt()
                nc.vector.tensor_scalar(c1[:], v1b[:], 16, None, LSR)
                v2a = wt()
                nc.vector.tensor_scalar(v2a[:], t01[:], 16, None, LSR)
                v2 = wt()
                nc.vector.tensor_tensor(v2[:], v2a[:], c1[:], ADD)
                L2a = wt()
                nc.vector.tensor_scalar(L2a[:], t11[:], 8, None, LSR)
                L2 = wt()
                nc.vector.tensor_tensor(L2[:], L2a[:], v2[:], ADD)

                # xor across the 3 terms -> H limbs [P, C]
                H = []
                for Lt in (L0, L1, L2):
                    Ht = st()
                    nc.vector.tensor_tensor(Ht[:], Lt[:, sl[0]], Lt[:, sl[1]],
                                            XOR)
                    nc.vector.tensor_tensor(Ht[:], Ht[:], Lt[:, sl[2]], XOR)
                    H.append(Ht)
                H0, H1, H2 = H

                # R-piece products, batched 4-wide: [H1a*R16 | H1b*R24 |
                # H2a*R32 | H2b*R40] then one mod pass against M4/I4
                pc = wp.tile([P, 4 * C], i32, tag="pc", bufs=2,
                             name=f"pc_{nc.next_id()}")
                H1a = st()
                nc.vector.tensor_scalar(H1a[:], H1[:], 0xFF, None, AND)
                H1b = st()
                nc.vector.tensor_scalar(H1b[:], H1[:], 8, None, LSR)
                H2a = st()
                nc.vector.tensor_scalar(H2a[:], H2[:], 0xFF, None, AND)
                H2b = st()
                nc.vector.tensor_scalar(H2b[:], H2[:], 8, None, LSR)
                for j, (piece, R) in enumerate(
                    ((H1a, R16), (H1b, R24), (H2a, R32), (H2b, R40))
                ):
                    nc.vector.tensor_tensor(pc[:, j * C : (j + 1) * C],
                                            piece[:], R, MULT)

                def wst(dtype=i32):
                    return wp.tile([P, 4 * C], dtype, tag=f"w4{dtype}",
                                   bufs=6, name=f"w4_{nc.next_id()}")

                y = wst(f32)
                nc.vector.tensor_tensor(y[:], pc[:], I4, MULT)
                y2 = wst(f32)
                nc.vector.tensor_scalar(y2[:], y[:], 0.5, None, SUB)
                q_ = wst()
                nc.vector.tensor_copy(q_[:], y2[:])
                qm = wst()
                nc.vector.tensor_tensor(qm[:], q_[:], M4, MULT)
                rr = wst()
                nc.vector.tensor_tensor(rr[:], pc[:], qm[:], SUB)
                ge = wst()
                nc.vector.tensor_tensor(ge[:], rr[:], M4, GE)
                gm = wst()
                nc.vector.tensor_tensor(gm[:], ge[:], M4, MULT)
                ps = wst()
                nc.vector.tensor_tensor(ps[:], rr[:], gm[:], SUB)

                x1 = st()
                nc.vector.tensor_tensor(x1[:], H0[:], ps[:, 0:C], ADD)
                x2 = st()
                nc.vector.tensor_tensor(x2[:], ps[:, C : 2 * C],
                                        ps[:, 2 * C : 3 * C], ADD)
                x3 = st()
                nc.vector.tensor_tensor(x3[:], x1[:], x2[:], ADD)
                x4 = st()
                nc.vector.tensor_tensor(x4[:], x3[:], ps[:, 3 * C :], ADD)

                # final mod -> [0, m), then += adj -> int16
                y5 = st(f32)
                nc.vector.tensor_tensor(y5[:], x4[:], INV, MULT)
                y6 = st(f32)
                nc.vector.tensor_scalar(y6[:], y5[:], 0.5, None, SUB)
                q6 = st()
                nc.vector.tensor_copy(q6[:], y6[:])
                qm6 = st()
                nc.vector.tensor_tensor(qm6[:], q6[:], Mt, MULT)
                r6 = st()
                nc.vector.tensor_tensor(r6[:], x4[:], qm6[:], SUB)
                ge6 = st()
                nc.vector.tensor_tensor(ge6[:], r6[:], Mt, GE)
                gm6 = st()
                nc.vector.tensor_tensor(gm6[:], ge6[:], Mt, MULT)
                r7 = st()
                nc.vector.tensor_tensor(r7[:], r6[:], gm6[:], SUB)
                ra = st()
                nc.vector.tensor_tensor(ra[:], r7[:], adj[:, cs], ADD)
                c16 = wp.tile([P, C], dt.int16, tag="c16", bufs=NSLAB,
                              name=f"c16_{sb}")
                nc.vector.tensor_copy(c16[:], ra[:])
                return c16

            c16s = [hash_slab(sb) for sb in range(NSLAB)]

            for a in range(NCHUNK):
                c16 = c16s[a // CPS]
                csl = c16[:, CW * (a % CPS) : CW * (a % CPS) + CW]
                # scramble [128, 32] -> [16, 256] wrapped stream layout for
                # queue q's tx/rx descriptor-gen partition groups
                q = a % 4
                stg = gp.tile([P, CPC // 16], dt.int16, tag=f"stg{q}",
                              bufs=2, name=f"stg{a}")
                nc.sync.dma_start(stg[32 * q : 32 * q + 16, :], csl)
                nc.sync.dma_start(stg[32 * q + 16 : 32 * q + 32, :], csl)
                d = gp.tile([P, CW * EM], f32, tag="d", bufs=5,
                            name=f"d_{a}")
                nc.gpsimd.dma_gather(
                    d[:].rearrange("p (b e) -> p b e", e=EM),
                    tblM.ap()[bases[a] :],
                    stg[:],
                    CPC,
                    CPC,
                    EM,
                    single_packet=False,
                    queue_num=q,
                )
                nc.sync.dma_start(
                    outMv[a],
                    d[:].rearrange("p (b e) -> p b e", e=EM)[:, :, :SLOT],
                )

    nc.compile()
    return nc


def _get_nc(bases):
    key = tuple(bases)
    if key not in _NC:
        _NC[key] = _build_nc(key)
    return _NC[key]


def _host_hashes(inputs):
    ids = np.asarray(inputs["canonical_ids"]).astype(np.int64)  # [B, S]
    hs = np.asarray(inputs["hash_seeds"]).astype(np.int64)      # [3, 8]
    cur = ids.reshape(-1)
    prv = np.pad(ids, ((0, 0), (1, 0)))[:, :S].reshape(-1)
    pv2 = np.pad(ids, ((0, 0), (2, 0)))[:, :S].reshape(-1)
    return cur, prv, pv2, hs


def _make_core(cur, prv, pv2, hs, inputs, c):
    s0, s1, s2 = int(hs[0, c]), int(hs[1, c]), int(hs[2, c])
    m2, m3 = N2_SIZES[c], N3_SIZES[c]

    with np.errstate(over="ignore"):
        h2 = ((prv * s0) ^ (cur * s1)) % m2
        h3 = ((pv2 * s0) ^ (prv * s1) ^ (cur * s2)) % m3
    vm = np.concatenate([h2, V2 + h3])          # [NE] merged-table indices
    order = np.argsort(vm)                       # sorted entry ids
    svm = vm[order]

    bases = np.array(DEFAULT_BASES, np.int64)
    rel = svm - np.repeat(bases, CPC)
    if not ((rel >= 0) & (rel < 32768)).all():
        # pathological hash skew: fall back to exact chunk-min bases
        bases = svm[:: CPC].copy()
        rel = svm - np.repeat(bases, CPC)
        assert ((rel >= 0) & (rel < 32768)).all()

    # sorted position j -> hash plane position (p, c) and outM row
    j = np.arange(NE)
    k = j // CPC
    i = j % CPC
    n = (i % 16) * (CPC // 16) + i // 16   # stg stream scramble bijection
    p = n // CW
    cc = n % CW
    col = CW * k + cc
    row = CPC * k + (i % 128) * CW + i // 128

    e = order
    isn3 = e >= NTOK
    t = e % NTOK
    id1 = prv[t]
    id2 = cur[t]
    id0 = np.where(isn3, pv2[t], 0)
    sd0 = np.where(isn3, s0, 0)
    sd1 = np.where(isn3, s1, s0)
    sd2 = np.where(isn3, s2, s1)
    m = np.where(isn3, m3, m2).astype(np.int64)
    tb = np.where(isn3, V2, 0)
    adjv = tb - bases[k]

    def plane(vals, dtype=np.int32):
        pl = np.empty((P, COLS), dtype)
        pl[p, col] = vals
        return pl

    def slab_blocked(planes):
        """[nterm, P, COLS] planes -> [P, nterm*COLS] slab-blocked."""
        nt = len(planes)
        out = np.empty((P, nt * COLS), planes[0].dtype)
        for sb in range(NSLAB):
            for term in range(nt):
                out[:, nt * SLABW * sb + term * SLABW :
                    nt * SLABW * sb + (term + 1) * SLABW] = (
                    planes[term][:, SLABW * sb : SLABW * (sb + 1)]
                )
        return out

    s0w = slab_blocked([plane(sd & 0xFFFF) for sd in (sd0, sd1, sd2)])
    s1w = slab_blocked([plane(sd >> 16) for sd in (sd0, sd1, sd2)])
    cst = np.empty((P, 5 * COLS), np.int32)
    for ri, rv in enumerate((m, 2**16 % m, 2**24 % m, 2**32 % m,
                             2**40 % m)):
        cst[:, ri * COLS : (ri + 1) * COLS] = plane(rv)
    mp = plane(m)
    invp = plane((1.0 / m) * (1 - 1e-6), np.float32)
    m4 = slab_blocked([mp, mp, mp, mp])
    i4 = slab_blocked([invp] * 4)
    idsP = np.stack([plane(id0).reshape(-1), plane(id1).reshape(-1),
                     plane(id2).reshape(-1)])

    tblM = np.zeros((VM, EM), np.float32)
    w2 = np.asarray(inputs[f"w_n2_h{c}"], dtype=np.float32)
    tblM[: w2.shape[0], :SLOT] = w2
    w3 = np.asarray(inputs[f"w_n3_h{c}"], dtype=np.float32)
    tblM[V2 : V2 + w3.shape[0], :SLOT] = w3

    rowse = np.empty(NE, np.int64)
    rowse[order] = row                     # entry id -> outM row

    in_map = {
        "tblM": tblM,
        "ids": np.ascontiguousarray(idsP.astype(np.int32)),
        "s0w": s0w,
        "s1w": s1w,
        "cst": cst,
        "inv": invp,
        "m4": m4,
        "i4": i4,
        "adj": plane(adjv),
    }
    return in_map, tuple(int(b) for b in bases), rowse


def kernel(**inputs):
    global LAST_RESULT
    cur, prv, pv2, hs = _host_hashes(inputs)
    in_maps, bases_l, rows_l = [], [], []
    for c in range(8):
        im, bs, rowse = _make_core(cur, prv, pv2, hs, inputs, c)
        in_maps.append(im)
        bases_l.append(bs)
        rows_l.append(rowse)
    assert all(b == bases_l[0] for b in bases_l[1:]), (
        "per-core base fallback mismatch; build per-core kernels instead"
    )
    nc = _get_nc(bases_l[0])
    res = run_bass_kernel_spmd(nc, in_maps, core_ids=list(range(8)),
                               trace=TRACE)
    LAST_RESULT = res
    out = np.empty((B, S, 16 * SLOT), np.float32)
    for c in range(8):
        oM = res.results[c]["outM"]                       # [NE, SLOT]
        rowse = rows_l[c]
        out[:, :, c * SLOT : (c + 1) * SLOT] = (
            oM[rowse[:NTOK]].reshape(B, S, SLOT)
        )
        out[:, :, (8 + c) * SLOT : (9 + c) * SLOT] = (
            oM[rowse[NTOK:]].reshape(B, S, SLOT)
        )
    return out


# revision 12
# speedup vs baseline: 1.0170x; 1.0170x over previous
"""EngramEmbeddings Trainium2 kernel — unified sorted-gather design.

Expert-sharded across 8 NeuronCores: core c owns head c of the n=2 and n=3
hash tables and serves all 32768 tokens for its two slots (65536 lookups).

Design:
  The n2 (6689-row) and n3 (65579-row) tables are concatenated into one
  merged table of 72268 rows, each padded to 512B (128 f32).  All 65536
  lookups (32768 n2 + 32768 n3) form one stream, HOST-sorted by index
  value into 16 chunks of 4096 whose index span fits int16 after
  subtracting a fixed per-chunk base (the sorted halves span ~840/~8.2k
  per chunk, far under 32768).  The host computes hashes ONLY to choose
  this permutation + verify the bases; the device recomputes the exact
  hash itself.

  Device phases (serialized on purpose — SWDGE descriptor generation
  starves the DVE of SBUF ports, so overlapping hash with gathers
  stretches vector ops up to ~100x):
    1. hash: unified 3-term hash (n2 entries use id0=0 so term0 == 0,
       XOR identity) over 4 slabs of 128 columns, with per-position
       seed/mod-constant planes; exact int64 semantics via 16-bit limb
       arithmetic on the vector engine; the four R-piece mods run as one
       4-wide batched mod.  += adj plane -> int16 idx in [0, 32768).
    2. gather: per chunk, two SBUF->SBUF copies scramble idx into the
       dma_gather wrapped stream layout for queue q's tx/rx partition
       groups (partitions 32q..32q+32), then dma_gather on queues 0-3
       round-robin (4 queues -> ~2ns/row of GpSimd desc-gen, the 4
       tx/rx Q7 core pairs all busy).  A dummy sync-engine copy after
       the last slab orders every stg copy (hence gather) behind the
       full hash.
    3. store: HWDGE (scalar-engine queue) writes the first 80 f32 of
       each gathered 128-f32 row -> outM[4096a..].

Host does sharding-style prep: dtype casts, the sort permutation and its
inverse, per-position constant planes, table concat/pad, final unpermute.
"""

import numpy as np

try:
    import concourse  # noqa: F401
except ImportError:  # pragma: no cover
    import sys

    for _p in ("/opt/trn_rl_repo", "/root/.axon_site/_ro/trn_rl_repo"):
        if _p not in sys.path:
            sys.path.insert(0, _p)

import concourse.tile as tile
from concourse import bacc, mybir
from concourse.bass_utils import run_bass_kernel_spmd

N2_SIZES = [6619, 6637, 6653, 6659, 6661, 6673, 6679, 6689]
N3_SIZES = [65521, 65537, 65539, 65543, 65551, 65557, 65563, 65579]
B, S = 8, 4096
P = 128
NTOK = B * S                  # 32768
NE = 2 * NTOK                 # 65536 lookups (n2 + n3)
COLS = NE // P                # 512 hash columns per partition
V2 = max(N2_SIZES)            # 6689
V3 = max(N3_SIZES)            # 65579
VM = V2 + V3                  # 72268 merged rows
EM = 128                      # merged row padded to 128 f32 = 512B
SLOT = 80
NCHUNK = 16                   # dma_gather chunks
CPC = NE // NCHUNK            # 4096 lookups per chunk
CW = CPC // P                 # 32 hash cols / sbuf row-blocks per chunk
SLABW = 128                   # hash slab width = 4 chunks
NSLAB = COLS // SLABW         # 4
CPS = SLABW // CW             # 4 chunks per slab

# fixed per-chunk bases: the sorted merged stream is exactly half n2
# (indices in [0, V2)) then half n3 ([V2, VM)), so chunks 0-7 are n2 and
# 8-15 n3; within each half chunks span ~1/8 of the range.  The -2048
# slack absorbs order-statistic fluctuation; host asserts every
# idx-base lands in [0, 32768) and recompiles with exact bases if not.
DEFAULT_BASES = tuple(
    max(0, (V2 * k) // 8 - 2048) if k < 8
    else max(0, V2 + (V3 * (k - 8)) // 8 - 2048)
    for k in range(NCHUNK)
)

_NC = {}
TRACE = False
LAST_RESULT = None


def _build_nc(bases):
    dt = mybir.dt
    A = mybir.AluOpType
    AND, XOR = A.bitwise_and, A.bitwise_xor
    LSR, LSL = A.logical_shift_right, A.logical_shift_left
    ADD, MULT, SUB, GE = A.add, A.mult, A.subtract, A.is_ge
    i32 = dt.int32
    f32 = dt.float32

    nc = bacc.Bacc("TRN2", target_bir_lowering=False, debug=False,
                   num_swdge_queues=4)
    tblM = nc.dram_tensor("tblM", [VM, EM], f32, kind="ExternalInput")
    idsd = nc.dram_tensor("ids", [3, NE], i32, kind="ExternalInput")
    # seed planes, slab-blocked: slab sb occupies cols [3*SLABW*sb,
    # 3*SLABW*(sb+1)) as [term0 | term1 | term2] blocks of SLABW
    s0d = nc.dram_tensor("s0w", [P, 3 * COLS], i32, kind="ExternalInput")
    s1d = nc.dram_tensor("s1w", [P, 3 * COLS], i32, kind="ExternalInput")
    # per-position mod constants: M, R16, R24, R32, R40 (int32) + inv (f32)
    cstd = nc.dram_tensor("cst", [P, 5 * COLS], i32, kind="ExternalInput")
    invd = nc.dram_tensor("inv", [P, COLS], f32, kind="ExternalInput")
    # 4x column-replicated M / inv for the batched piece-mod, slab-blocked:
    # slab sb at [4*SLABW*sb, 4*SLABW*(sb+1)), four identical SLABW blocks
    m4d = nc.dram_tensor("m4", [P, 4 * COLS], i32, kind="ExternalInput")
    i4d = nc.dram_tensor("i4", [P, 4 * COLS], f32, kind="ExternalInput")
    adjd = nc.dram_tensor("adj", [P, COLS], i32, kind="ExternalInput")
    outMd = nc.dram_tensor("outM", [NE, SLOT], f32, kind="ExternalOutput")

    with tile.TileContext(nc) as tc:
        with (
            tc.tile_pool(name="c", bufs=1) as cp,
            tc.tile_pool(name="w", bufs=1) as wp,
            tc.tile_pool(name="g", bufs=1) as gp,
        ):

            def ld(dram, shape, dtype, tag):
                t = cp.tile(shape, dtype, tag=tag, name=tag)
                nc.sync.dma_start(t[:], dram.ap())
                return t

            s0w = ld(s0d, [P, 3 * COLS], i32, "s0w")
            s1w = ld(s1d, [P, 3 * COLS], i32, "s1w")
            cst = ld(cstd, [P, 5 * COLS], i32, "cst")
            inv = ld(invd, [P, COLS], f32, "inv")
            m4 = ld(m4d, [P, 4 * COLS], i32, "m4")
            i4 = ld(i4d, [P, 4 * COLS], f32, "i4")
            adj = ld(adjd, [P, COLS], i32, "adj")

            idsv = idsd.ap().rearrange("r (p c) -> r p c", p=P)
            ids = []
            for r in range(3):
                t_ = cp.tile([P, COLS], i32, tag=f"id{r}", name=f"id{r}")
                nc.sync.dma_start(t_[:], idsv[r])
                ids.append(t_)

            outMv = outMd.ap().rearrange("(k p b) d -> k p b d", p=P, b=CW)

            def hash_slab(sb):
                """int16 gather indices for slab sb (4 chunks, 128 cols).

                Exact int64 hash via 16-bit limbs (DVE fp32-internal ops
                stay < 2^24; bit surgery uses exact int32 bitwise/shifts;
                mod is reciprocal-multiply + floor + conditional subtract).
                """
                C = SLABW
                W = 3 * C
                col0 = C * sb

                def wt():
                    return wp.tile([P, W], i32, tag="wm", bufs=14,
                                   name=f"wm_{nc.next_id()}")

                def st(dtype=i32):
                    return wp.tile([P, C], dtype, tag=f"sm{dtype}",
                                   bufs=10, name=f"sm_{nc.next_id()}")

                sl = [slice(j * C, (j + 1) * C) for j in range(3)]
                cs = slice(col0, col0 + C)
                Mt = cst[:, 0 * COLS + col0 : 0 * COLS + col0 + C]
                R16 = cst[:, 1 * COLS + col0 : 1 * COLS + col0 + C]
                R24 = cst[:, 2 * COLS + col0 : 2 * COLS + col0 + C]
                R32 = cst[:, 3 * COLS + col0 : 3 * COLS + col0 + C]
                R40 = cst[:, 4 * COLS + col0 : 4 * COLS + col0 + C]
                INV = inv[:, cs]
                s0v = s0w[:, 3 * col0 : 3 * col0 + W]
                s1v = s1w[:, 3 * col0 : 3 * col0 + W]
                M4 = m4[:, 4 * col0 : 4 * col0 + 4 * C]
                I4 = i4[:, 4 * col0 : 4 * col0 + 4 * C]

                X = wt()
                for j, src in enumerate(ids):
                    nc.scalar.copy(X[:, sl[j]], src[:, cs])
                a0 = wt()
                nc.vector.tensor_scalar(a0[:], X[:], 0xFF, None, AND)
                a1 = wt()
                nc.vector.tensor_scalar(a1[:], X[:], 8, None, LSR)
                t00, t10, t01, t11 = wt(), wt(), wt(), wt()
                nc.vector.tensor_tensor(t00[:], a0[:], s0v, MULT)
                nc.vector.tensor_tensor(t10[:], a1[:], s0v, MULT)
                nc.vector.tensor_tensor(t01[:], a0[:], s1v, MULT)
                nc.vector.tensor_tensor(t11[:], a1[:], s1v, MULT)
                Apt = wt()
                nc.vector.tensor_scalar(Apt[:], t10[:], 0xFF, 8, AND, LSL)
                v0a = wt()
                nc.vector.tensor_scalar(v0a[:], t00[:], 0xFFFF, None, AND)
                v0 = wt()
                nc.vector.tensor_tensor(v0[:], v0a[:], Apt[:], ADD)
                L0 = wt()
                nc.vector.tensor_scalar(L0[:], v0[:], 0xFFFF, None, AND)
                c0 = wt()
                nc.vector.tensor_scalar(c0[:], v0[:], 16, None, LSR)
                u1a = wt()
                nc.vector.tensor_scalar(u1a[:], t10[:], 8, None, LSR)
                u1 = wt()
                nc.vector.tensor_tensor(u1[:], u1a[:], c0[:], ADD)
                u2a = wt()
                nc.vector.tensor_scalar(u2a[:], t01[:], 0xFFFF, None, AND)
                u2 = wt()
                nc.vector.tensor_tensor(u2[:], u2a[:], u1[:], ADD)
                u3a = wt()
                nc.vector.tensor_scalar(u3a[:], t00[:], 16, None, LSR)
                v1 = wt()
                nc.vector.tensor_tensor(v1[:], u3a[:], u2[:], ADD)
                Ff = wt()
                nc.vector.tensor_scalar(Ff[:], t11[:], 0xFF, 8, AND, LSL)
                v1b = wt()
                nc.vector.tensor_tensor(v1b[:], v1[:], Ff[:], ADD)
                L1 = wt()
                nc.vector.tensor_scalar(L1[:], v1b[:], 0xFFFF, None, AND)
                c1 = wt()
                nc.vector.tensor_scalar(c1[:], v1b[:], 16, None, LSR)
                v2a = wt()
                nc.vector.tensor_scalar(v2a[:], t01[:], 16, None, LSR)
                v2 = wt()
                nc.vector.tensor_tensor(v2[:], v2a[:], c1[:], ADD)
                L2a = wt()
                nc.vector.tensor_scalar(L2a[:], t11[:], 8, None, LSR)
                L2 = wt()
                nc.vector.tensor_tensor(L2[:], L2a[:], v2[:], ADD)

                # xor across the 3 terms -> H limbs [P, C]
                H = []
                for Lt in (L0, L1, L2):
                    Ht = st()
                    nc.vector.tensor_tensor(Ht[:], Lt[:, sl[0]], Lt[:, sl[1]],
                                            XOR)
                    nc.vector.tensor_tensor(Ht[:], Ht[:], Lt[:, sl[2]], XOR)
                    H.append(Ht)
                H0, H1, H2 = H

                # R-piece products, batched 4-wide: [H1a*R16 | H1b*R24 |
                # H2a*R32 | H2b*R40] then one mod pass against M4/I4
                pc = wp.tile([P, 4 * C], i32, tag="pc", bufs=2,
                             name=f"pc_{nc.next_id()}")
                H1a = st()
                nc.vector.tensor_scalar(H1a[:], H1[:], 0xFF, None, AND)
                H1b = st()
                nc.vector.tensor_scalar(H1b[:], H1[:], 8, None, LSR)
                H2a = st()
                nc.vector.tensor_scalar(H2a[:], H2[:], 0xFF, None, AND)
                H2b = st()
                nc.vector.tensor_scalar(H2b[:], H2[:], 8, None, LSR)
                for j, (piece, R) in enumerate(
                    ((H1a, R16), (H1b, R24), (H2a, R32), (H2b, R40))
                ):
                    nc.vector.tensor_tensor(pc[:, j * C : (j + 1) * C],
                                            piece[:], R, MULT)

                def wst(dtype=i32):
                    return wp.tile([P, 4 * C], dtype, tag=f"w4{dtype}",
                                   bufs=6, name=f"w4_{nc.next_id()}")

                y = wst(f32)
                nc.vector.tensor_tensor(y[:], pc[:], I4, MULT)
                y2 = wst(f32)
                nc.vector.tensor_scalar(y2[:], y[:], 0.5, None, SUB)
                q_ = wst()
                nc.vector.tensor_copy(q_[:], y2[:])
                qm = wst()
                nc.vector.tensor_tensor(qm[:], q_[:], M4, MULT)
                rr = wst()
                nc.vector.tensor_tensor(rr[:], pc[:], qm[:], SUB)
                ge = wst()
                nc.vector.tensor_tensor(ge[:], rr[:], M4, GE)
                gm = wst()
                nc.vector.tensor_tensor(gm[:], ge[:], M4, MULT)
                ps = wst()
                nc.vector.tensor_tensor(ps[:], rr[:], gm[:], SUB)

                x1 = st()
                nc.vector.tensor_tensor(x1[:], H0[:], ps[:, 0:C], ADD)
                x2 = st()
                nc.vector.tensor_tensor(x2[:], ps[:, C : 2 * C],
                                        ps[:, 2 * C : 3 * C], ADD)
                x3 = st()
                nc.vector.tensor_tensor(x3[:], x1[:], x2[:], ADD)
                x4 = st()
                nc.vector.tensor_tensor(x4[:], x3[:], ps[:, 3 * C :], ADD)

                # final mod -> [0, m), then += adj -> int16
                y5 = st(f32)
                nc.vector.tensor_tensor(y5[:], x4[:], INV, MULT)
                y6 = st(f32)
                nc.vector.tensor_scalar(y6[:], y5[:], 0.5, None, SUB)
                q6 = st()
                nc.vector.tensor_copy(q6[:], y6[:])
                qm6 = st()
                nc.vector.tensor_tensor(qm6[:], q6[:], Mt, MULT)
                r6 = st()
                nc.vector.tensor_tensor(r6[:], x4[:], qm6[:], SUB)
                ge6 = st()
                nc.vector.tensor_tensor(ge6[:], r6[:], Mt, GE)
                gm6 = st()
                nc.vector.tensor_tensor(gm6[:], ge6[:], Mt, MULT)
                r7 = st()
                nc.vector.tensor_tensor(r7[:], r6[:], gm6[:], SUB)
                ra = st()
                nc.vector.tensor_tensor(ra[:], r7[:], adj[:, cs], ADD)
                c16 = wp.tile([P, C], dt.int16, tag="c16", bufs=NSLAB,
                              name=f"c16_{sb}")
                nc.vector.tensor_copy(c16[:], ra[:])
                return c16

            c16s = [hash_slab(sb) for sb in range(NSLAB)]

            # dummy sync-engine op depending on the LAST slab: every stg
            # copy queues behind it on the sync engine, so no gather
            # desc-gen (SBUF-port hog) starts until the whole hash is done.
            scratch = gp.tile([P, 1], dt.int16, tag="scr", name="scr")
            nc.sync.dma_start(scratch[:], c16s[-1][:, 0:1])

            for a in range(NCHUNK):
                c16 = c16s[a // CPS]
                csl = c16[:, CW * (a % CPS) : CW * (a % CPS) + CW]
                # scramble [128, 32] -> [16, 256] wrapped stream layout for
                # queue q's tx/rx descriptor-gen partition groups
                q = a % 4
                stg = gp.tile([P, CPC // 16], dt.int16, tag=f"stg{q}",
                              bufs=4, name=f"stg{a}")
                nc.sync.dma_start(stg[32 * q : 32 * q + 16, :], csl)
                nc.sync.dma_start(stg[32 * q + 16 : 32 * q + 32, :], csl)
                d = gp.tile([P, CW * EM], f32, tag="d", bufs=5,
                            name=f"d_{a}")
                nc.gpsimd.dma_gather(
                    d[:].rearrange("p (b e) -> p b e", e=EM),
                    tblM.ap()[bases[a] :],
                    stg[:],
                    CPC,
                    CPC,
                    EM,
                    single_packet=False,
                    queue_num=q,
                )
                nc.sync.dma_start(
                    outMv[a],
                    d[:].rearrange("p (b e) -> p b e", e=EM)[:, :, :SLOT],
                )

    nc.compile()
    return nc


def _get_nc(bases):
    key = tuple(bases)
    if key not in _NC:
        _NC[key] = _build_nc(key)
    return _NC[key]


def _host_hashes(inputs):
    ids = np.asarray(inputs["canonical_ids"]).astype(np.int64)  # [B, S]
    hs = np.asarray(inputs["hash_seeds"]).astype(np.int64)      # [3, 8]
    cur = ids.reshape(-1)
    prv = np.pad(ids, ((0, 0), (1, 0)))[:, :S].reshape(-1)
    pv2 = np.pad(ids, ((0, 0), (2, 0)))[:, :S].reshape(-1)
    return cur, prv, pv2, hs


def _make_core(cur, prv, pv2, hs, inputs, c):
    s0, s1, s2 = int(hs[0, c]), int(hs[1, c]), int(hs[2, c])
    m2, m3 = N2_SIZES[c], N3_SIZES[c]

    with np.errstate(over="ignore"):
        h2 = ((prv * s0) ^ (cur * s1)) % m2
        h3 = ((pv2 * s0) ^ (prv * s1) ^ (cur * s2)) % m3
    vm = np.concatenate([h2, V2 + h3])          # [NE] merged-table indices
    order = np.argsort(vm)                       # sorted entry ids
    svm = vm[order]

    bases = np.array(DEFAULT_BASES, np.int64)
    rel = svm - np.repeat(bases, CPC)
    if not ((rel >= 0) & (rel < 32768)).all():
        # pathological hash skew: fall back to exact chunk-min bases
        bases = svm[:: CPC].copy()
        rel = svm - np.repeat(bases, CPC)
        assert ((rel >= 0) & (rel < 32768)).all()

    # sorted position j -> hash plane position (p, c) and outM row
    j = np.arange(NE)
    k = j // CPC
    i = j % CPC
    n = (i % 16) * (CPC // 16) + i // 16   # stg stream scramble bijection
    p = n // CW
    cc = n % CW
    col = CW * k + cc
    row = CPC * k + (i % 128) * CW + i // 128

    e = order
    isn3 = e >= NTOK
    t = e % NTOK
    id1 = prv[t]
    id2 = cur[t]
    id0 = np.where(isn3, pv2[t], 0)
    sd0 = np.where(isn3, s0, 0)
    sd1 = np.where(isn3, s1, s0)
    sd2 = np.where(isn3, s2, s1)
    m = np.where(isn3, m3, m2).astype(np.int64)
    tb = np.where(isn3, V2, 0)
    adjv = tb - bases[k]

    def plane(vals, dtype=np.int32):
        pl = np.empty((P, COLS), dtype)
        pl[p, col] = vals
        return pl

    def slab_blocked(planes):
        """[nterm, P, COLS] planes -> [P, nterm*COLS] slab-blocked."""
        nt = len(planes)
        out = np.empty((P, nt * COLS), planes[0].dtype)
        for sb in range(NSLAB):
            for term in range(nt):
                out[:, nt * SLABW * sb + term * SLABW :
                    nt * SLABW * sb + (term + 1) * SLABW] = (
                    planes[term][:, SLABW * sb : SLABW * (sb + 1)]
                )
        return out

    s0w = slab_blocked([plane(sd & 0xFFFF) for sd in (sd0, sd1, sd2)])
    s1w = slab_blocked([plane(sd >> 16) for sd in (sd0, sd1, sd2)])
    cst = np.empty((P, 5 * COLS), np.int32)
    for ri, rv in enumerate((m, 2**16 % m, 2**24 % m, 2**32 % m,
                             2**40 % m)):
        cst[:, ri * COLS : (ri + 1) * COLS] = plane(rv)
    mp = plane(m)
    invp = plane((1.0 / m) * (1 - 1e-6), np.float32)
    m4 = slab_blocked([mp, mp, mp, mp])
    i4 = slab_blocked([invp] * 4)
    idsP = np.stack([plane(id0).reshape(-1), plane(id1).reshape(-1),
                     plane(id2).reshape(-1)])

    tblM = np.zeros((VM, EM), np.float32)
    w2 = np.asarray(inputs[f"w_n2_h{c}"], dtype=np.float32)
    tblM[: w2.shape[0], :SLOT] = w2
    w3 = np.asarray(inputs[f"w_n3_h{c}"], dtype=np.float32)
    tblM[V2 : V2 + w3.shape[0], :SLOT] = w3

    rowse = np.empty(NE, np.int64)
    rowse[order] = row                     # entry id -> outM row

    in_map = {
        "tblM": tblM,
        "ids": np.ascontiguousarray(idsP.astype(np.int32)),
        "s0w": s0w,
        "s1w": s1w,
        "cst": cst,
        "inv": invp,
        "m4": m4,
        "i4": i4,
        "adj": plane(adjv),
    }
    return in_map, tuple(int(b) for b in bases), rowse


def kernel(**inputs):
    global LAST_RESULT
    cur, prv, pv2, hs = _host_hashes(inputs)
    in_maps, bases_l, rows_l = [], [], []
    for c in range(8):
        im, bs, rowse = _make_core(cur, prv, pv2, hs, inputs, c)
        in_maps.append(im)
        bases_l.append(bs)
        rows_l.append(rowse)
    assert all(b == bases_l[0] for b in bases_l[1:]), (
        "per-core base fallback mismatch; build per-core kernels instead"
    )
    nc = _get_nc(bases_l[0])
    res = run_bass_kernel_spmd(nc, in_maps, core_ids=list(range(8)),
                               trace=TRACE)
    LAST_RESULT = res
    out = np.empty((B, S, 16 * SLOT), np.float32)
    for c in range(8):
        oM = res.results[c]["outM"]                       # [NE, SLOT]
        rowse = rows_l[c]
        out[:, :, c * SLOT : (c + 1) * SLOT] = (
            oM[rowse[:NTOK]].reshape(B, S, SLOT)
        )
        out[:, :, (8 + c) * SLOT : (9 + c) * SLOT] = (
            oM[rowse[NTOK:]].reshape(B, S, SLOT)
        )
    return out
